# revision 11
# baseline (speedup 1.0000x reference)
"""Trainium2 Bass kernel: 3x3 valid conv (64ch -> 128ch) + per-pixel bias.

Strategy: shard the 510 output rows spatially across 8 NeuronCores (64
rows/core with a 2-row input halo).  Inside a core, the 64-row band is
split across the two PE row-quadrants: partitions 0-63 hold the input
rows for output rows 0-31 of the band, partitions 64-127 the rows for
output rows 32-63, so the two 64x128 PE tiles stream concurrently.

Everything rides bf16 (input, weights, bias, output) to halve HBM
traffic vs fp32: 21.5 MB/core total, under the ~24 MB the matmul
stream window can absorb at ~385 GB/s.  The matmul stream itself is
the roofline: 288 pair-slots x 510 rows at 1 row/cycle.

Three stream optimizations:
 - LDWEIGHTS dedup: Tile legalization emits one LDWEIGHTS per matmul;
   each load steals 64 row-slots from the PE input bus.  Two
   consecutive rows share each tap's weights, and a post-legalize pass
   removes the duplicate loads (verified bit-exact on HW).
 - PE warmup: the PE ramps p-states over ~3us; eight dummy matmuls on
   a zeroed tile run while the weight DMA is in flight so the real
   stream starts at full clock.
 - Stores stay at 4-row granularity (4080B per-partition lines): the
   DMA rings are packet-rate-bound below ~4KB, so finer splits only
   lengthen the tail.

DMA schedule: weights first on the sync ring (the first matmul gates
on them), input chunks front-loaded on the scalar ring, all 16 bias
tiles resident (no WAR waits), stores g0-g5 on scalar / g6-g7 on sync.
"""

import numpy as np
from contextlib import ExitStack

import concourse.bass as bass
import concourse.tile as tile
from concourse import bacc, mybir
from concourse import bass_utils

C, H, W = 64, 512, 512
D, KK = 128, 3
OH, OW = H - KK + 1, W - KK + 1          # 510, 510
NCORES = 8
RPC = 64                                  # output rows per core
BAND = RPC + KK - 1                       # 66 input rows per core
HALF = RPC // 2                           # 32 output rows per strip
IBAND = HALF + KK - 1                     # 34 input rows per strip
NWAVE = HALF // 2                         # 16 waves of 2 rows
NGRP = HALF // 4                          # 8 groups of 4 rows
NDUMMY = 6                                # PE warmup matmuls
WCOL = 9 * D                              # weight columns in the packed tile

f32 = mybir.dt.float32
bf16 = mybir.dt.bfloat16

STARTS = [min(i * RPC, OH - RPC) for i in range(NCORES)]

_CACHE = {}
LAST_RESULTS = None


def _dedup_ldweights(nc):
    """Remove InstLdweights that reload the weights already resident in
    the same PE quadrant.  Runs after TileContext exit (legalization has
    inserted the loads; semaphore waits still live on the matmults, and
    duplicate loads carry no sync_info) and before nc.compile()."""
    removed = 0
    for fn in nc.m.functions:
        for bb in fn.blocks:
            last = {}
            keep = []
            for inst in bb.instructions:
                tn = type(inst).__name__
                if tn == "InstLdweights":
                    ap = inst.ins[0]
                    key = (str(inst.engine), tuple(inst.tile_position or (0, 0)))
                    sig = (ap.memref, ap.offset, str(ap.ap), str(ap.dtype))
                    si = inst.sync_info
                    clean = si is None or (
                        len(si.on_wait) == 0 and len(si.on_update) == 0
                    )
                    if last.get(key) == sig and clean:
                        removed += 1
                        continue
                    last[key] = sig
                elif tn == "InstMatmult":
                    pass
                else:
                    if tn not in ("InstTensorTensor", "InstActivation",
                                  "InstDMACopy", "InstTensorCopy",
                                  "InstMemset", "InstEventSemaphore"):
                        last = {}
                keep.append(inst)
            bb.instructions[:] = keep
    return removed


def _build_program():
    nc = bacc.Bacc(
        "TRN2", target_bir_lowering=False, debug=False, num_devices=NCORES
    )
    # x pre-split on host: row (h*64+c) holds strip-h band rows, flattened
    x = nc.dram_tensor("x", [2 * C, IBAND * W], bf16, kind="ExternalInput").ap()
    # w pre-duplicated (rows 0-63 == 64-127, [c, (ky kx d)]) with the first
    # three input band rows packed behind it: one sync-ring DMA delivers
    # everything the first wave's ky=0/1 taps need
    w = nc.dram_tensor(
        "w", [2 * C, WCOL + 3 * W], bf16, kind="ExternalInput"
    ).ap()
    b = nc.dram_tensor("b", [D, RPC, OW], bf16, kind="ExternalInput").ap()
    y = nc.dram_tensor("y", [D, RPC, OW], bf16, kind="ExternalOutput").ap()

    b_flat = b.rearrange("d r x -> d (r x)")
    y_flat = y.rearrange("d r x -> d (r x)")

    with tile.TileContext(nc) as tc:
        with ExitStack() as ctx:
            xp = ctx.enter_context(tc.tile_pool(name="xin", bufs=1))
            wp = ctx.enter_context(tc.tile_pool(name="wt", bufs=1))
            bp = ctx.enter_context(tc.tile_pool(name="bias", bufs=8))
            op = ctx.enter_context(tc.tile_pool(name="out", bufs=3))
            pp = ctx.enter_context(tc.tile_pool(name="ps", bufs=2, space="PSUM"))

            # PE warmup source: zeroed tile, no DMA dependency
            wdum = wp.tile([128, 640], bf16)
            nc.gpsimd.memset(wdum[:], 0.0)

            # weights + first two band rows gate the first real matmul:
            # first on the sync ring
            wt = wp.tile([128, WCOL + 3 * W], bf16)
            nc.sync.dma_start(wt[:], w[:, :])

            # rest of the input band, front-loaded on the scalar ring
            # (row 1 is re-loaded into xin: wave0's ky=1 taps read it there)
            xin = xp.tile([128, IBAND * W], bf16)
            bounds = [1, 5, 11, 19, 27, IBAND]
            for ci in range(len(bounds) - 1):
                r0, r1 = bounds[ci], bounds[ci + 1]
                nc.scalar.dma_start(xin[:, r0 * W:r1 * W], x[:, r0 * W:r1 * W])

            # all 16 bias tiles resident -> every load enqueues with no wait
            bias_tiles = []
            for g in range(NGRP):
                ra, rb = g * 4, HALF + g * 4
                ba = bp.tile([128, 4 * OW], bf16, name=f"ba{g}", tag="ba")
                nc.sync.dma_start(ba[:], b_flat[:, ra * OW:(ra + 4) * OW])
                bb = bp.tile([128, 4 * OW], bf16, name=f"bb{g}", tag="bb")
                nc.sync.dma_start(bb[:], b_flat[:, rb * OW:(rb + 4) * OW])
                bias_tiles.append((ba, bb))

            # warmup: keep the PE busy (and ramped) while weights land
            pdum = pp.tile([128, OW], f32, name="pdum", tag="pa0")
            for i in range(NDUMMY):
                nc.tensor.matmul(
                    pdum[:], wdum[0:128, 512:640], wdum[0:128, 0:510],
                    start=(i == 0), stop=(i == NDUMMY - 1),
                )

            ya = yb = None
            for wv in range(NWAVE):
                g, half = divmod(wv, 2)
                j0 = 2 * wv                    # strip-local output rows
                ba, bb = bias_tiles[g]
                if half == 0:
                    ya = op.tile([128, 4 * OW], bf16, name=f"ya{g}", tag="ya")
                    yb = op.tile([128, 4 * OW], bf16, name=f"yb{g}", tag="yb")

                pa0 = pp.tile([128, OW], f32, name="pa0", tag="pa0")
                pa1 = pp.tile([128, OW], f32, name="pa1", tag="pa1")
                pb0 = pp.tile([128, OW], f32, name="pb0", tag="pb0")
                pb1 = pp.tile([128, OW], f32, name="pb1", tag="pb1")
                for t in range(9):
                    ky, kx = divmod(t, 3)
                    if wv == 0 and ky < 2:
                        # rows 0-2 arrive packed behind the weights
                        src = wt
                        o0 = WCOL + ky * W + kx
                        o1 = WCOL + (ky + 1) * W + kx
                    else:
                        src = xin
                        o0 = (j0 + ky) * W + kx
                        o1 = (j0 + 1 + ky) * W + kx
                    st, sp = (t == 0), (t == 8)
                    ws = wt[0:64, t * D:(t + 1) * D]
                    nc.tensor.matmul(pa0[:], ws, src[0:64, o0:o0 + OW],
                                     start=st, stop=sp)
                    nc.tensor.matmul(pa1[:], ws, src[0:64, o1:o1 + OW],
                                     start=st, stop=sp)
                    ws = wt[64:128, t * D:(t + 1) * D]
                    nc.tensor.matmul(pb0[:], ws, src[64:128, o0:o0 + OW],
                                     start=st, stop=sp)
                    nc.tensor.matmul(pb1[:], ws, src[64:128, o1:o1 + OW],
                                     start=st, stop=sp)

                s0 = slice((2 * half) * OW, (2 * half + 1) * OW)
                s1 = slice((2 * half + 1) * OW, (2 * half + 2) * OW)
                if wv == NWAVE - 1:
                    # final wave: strip-b first so its ring starts draining
                    # while strip-a is still evacuating
                    nc.vector.tensor_add(yb[:, s0], pb0[:], bb[:, s0])
                    nc.vector.tensor_add(yb[:, s1], pb1[:], bb[:, s1])
                    nc.vector.tensor_add(ya[:, s0], pa0[:], ba[:, s0])
                    nc.vector.tensor_add(ya[:, s1], pa1[:], ba[:, s1])
                else:
                    nc.vector.tensor_add(ya[:, s0], pa0[:], ba[:, s0])
                    nc.vector.tensor_add(ya[:, s1], pa1[:], ba[:, s1])
                    nc.vector.tensor_add(yb[:, s0], pb0[:], bb[:, s0])
                    nc.vector.tensor_add(yb[:, s1], pb1[:], bb[:, s1])

                ra, rb = g * 4, HALF + g * 4
                if g < NGRP - 1:
                    if half == 1:
                        ea = nc.scalar if g < 6 else nc.sync
                        eb = nc.scalar
                        ea.dma_start(y_flat[:, ra * OW:(ra + 4) * OW], ya[:])
                        eb.dma_start(y_flat[:, rb * OW:(rb + 4) * OW], yb[:])
                else:
                    # last group: store each wave's 2 rows as soon as its
                    # evac lands, one strip per ring
                    r0 = (ra + 2 * half) * OW
                    r1 = (rb + 2 * half) * OW
                    ss = slice(2 * half * OW, (2 * half + 2) * OW)
                    nc.sync.dma_start(y_flat[:, r0:r0 + 2 * OW], ya[:, ss])
                    nc.scalar.dma_start(y_flat[:, r1:r1 + 2 * OW], yb[:, ss])

    ndedup = _dedup_ldweights(nc)
    assert ndedup >= 288, f"ldweights dedup removed only {ndedup}"
    nc.compile()
    return nc


def kernel(input, kernels, biases):
    global LAST_RESULTS
    import ml_dtypes
    if "nc" not in _CACHE:
        _CACHE["nc"] = _build_program()
    nc = _CACHE["nc"]

    xb = np.asarray(input, dtype=np.float32).astype(ml_dtypes.bfloat16)
    w1 = (
        np.ascontiguousarray(np.asarray(kernels, np.float32).transpose(1, 2, 3, 0))
        .reshape(C, 9 * D).astype(ml_dtypes.bfloat16)
    )
    wr = np.ascontiguousarray(np.concatenate([w1, w1], axis=0))
    bb = np.asarray(biases, np.float32).astype(ml_dtypes.bfloat16)

    in_maps = []
    for s in STARTS:
        band = xb[:, s:s + BAND, :]
        xs = np.concatenate(
            [band[:, 0:IBAND, :], band[:, HALF:HALF + IBAND, :]], axis=0
        ).reshape(2 * C, IBAND * W)
        wx = np.concatenate([wr, xs[:, 0:3 * W]], axis=1)
        in_maps.append({
            "x": np.ascontiguousarray(xs),
            "w": np.ascontiguousarray(wx),
            "b": np.ascontiguousarray(bb[:, s:s + RPC, :]),
        })

    res = bass_utils.run_bass_kernel_spmd(
        nc, in_maps, core_ids=list(range(NCORES))
    )
    LAST_RESULTS = res

    out = np.empty((D, OH, OW), np.float32)
    for i, s in enumerate(STARTS):
        out[:, s:s + RPC, :] = np.asarray(res.results[i]["y"], dtype=np.float32)
    return out


# revision 13
# speedup vs baseline: 1.1646x; 1.1646x over previous
"""Trainium2 Bass kernel: 3x3 valid conv (64ch -> 128ch) + per-pixel bias.

Strategy: shard the 510 output rows spatially across 8 NeuronCores (64
rows/core with a 2-row input halo).  Inside a core, the 64-row band is
split across the two PE row-quadrants: partitions 0-63 hold the input
rows for output rows 0-31 of the band, partitions 64-127 the rows for
output rows 32-63, so the two 64x128 PE tiles stream concurrently.

Everything rides bf16 (input, weights, bias, output) to halve HBM
traffic vs fp32: 21.5 MB/core total, under the ~24 MB the matmul
stream window can absorb at ~385 GB/s.  The matmul stream itself is
the roofline: 288 pair-slots x 510 rows at 1 row/cycle.

Three stream optimizations:
 - LDWEIGHTS dedup: Tile legalization emits one LDWEIGHTS per matmul;
   each load steals 64 row-slots from the PE input bus.  Two
   consecutive rows share each tap's weights, and a post-legalize pass
   removes the duplicate loads (verified bit-exact on HW).
 - PE warmup: the PE ramps p-states over ~3us; eight dummy matmuls on
   a zeroed tile run while the weight DMA is in flight so the real
   stream starts at full clock.
 - Stores stay at 4-row granularity (4080B per-partition lines): the
   DMA rings are packet-rate-bound below ~4KB, so finer splits only
   lengthen the tail.

DMA schedule: weights first on the sync ring (the first matmul gates
on them), input chunks front-loaded on the scalar ring, all 16 bias
tiles resident (no WAR waits), stores g0-g5 on scalar / g6-g7 on sync.
"""

import numpy as np
from contextlib import ExitStack

import concourse.bass as bass
import concourse.tile as tile
from concourse import bacc, mybir
from concourse import bass_utils

C, H, W = 64, 512, 512
D, KK = 128, 3
OH, OW = H - KK + 1, W - KK + 1          # 510, 510
NCORES = 8
RPC = 64                                  # output rows per core
BAND = RPC + KK - 1                       # 66 input rows per core
HALF = RPC // 2                           # 32 output rows per strip
IBAND = HALF + KK - 1                     # 34 input rows per strip
NWAVE = HALF // 2                         # 16 waves of 2 rows
NGRP = HALF // 4                          # 8 groups of 4 rows
NDUMMY = 8                                # PE warmup matmuls
WCOL = 9 * D                              # weight columns in the packed tile

f32 = mybir.dt.float32
bf16 = mybir.dt.bfloat16

STARTS = [min(i * RPC, OH - RPC) for i in range(NCORES)]

_CACHE = {}
LAST_RESULTS = None


def _dedup_ldweights(nc):
    """Remove InstLdweights that reload the weights already resident in
    the same PE quadrant.  Runs after TileContext exit (legalization has
    inserted the loads; semaphore waits still live on the matmults, and
    duplicate loads carry no sync_info) and before nc.compile()."""
    removed = 0
    for fn in nc.m.functions:
        for bb in fn.blocks:
            last = {}
            keep = []
            for inst in bb.instructions:
                tn = type(inst).__name__
                if tn == "InstLdweights":
                    ap = inst.ins[0]
                    key = (str(inst.engine), tuple(inst.tile_position or (0, 0)))
                    sig = (ap.memref, ap.offset, str(ap.ap), str(ap.dtype))
                    si = inst.sync_info
                    clean = si is None or (
                        len(si.on_wait) == 0 and len(si.on_update) == 0
                    )
                    if last.get(key) == sig and clean:
                        removed += 1
                        continue
                    last[key] = sig
                elif tn == "InstMatmult":
                    pass
                else:
                    if tn not in ("InstTensorTensor", "InstActivation",
                                  "InstDMACopy", "InstTensorCopy",
                                  "InstMemset", "InstEventSemaphore"):
                        last = {}
                keep.append(inst)
            bb.instructions[:] = keep
    return removed


def _build_program():
    nc = bacc.Bacc(
        "TRN2", target_bir_lowering=False, debug=False, num_devices=NCORES
    )
    # x pre-split on host: row (h*64+c) holds strip-h band rows, flattened
    x = nc.dram_tensor("x", [2 * C, IBAND * W], bf16, kind="ExternalInput").ap()
    # w pre-duplicated (rows 0-63 == 64-127, [c, (ky kx d)]) with the first
    # three input band rows packed behind it: one sync-ring DMA delivers
    # everything the first wave's ky=0/1 taps need
    w = nc.dram_tensor(
        "w", [2 * C, WCOL + 3 * W], bf16, kind="ExternalInput"
    ).ap()
    b = nc.dram_tensor("b", [D, RPC, OW], bf16, kind="ExternalInput").ap()
    y = nc.dram_tensor("y", [D, RPC, OW], bf16, kind="ExternalOutput").ap()

    b_flat = b.rearrange("d r x -> d (r x)")
    y_flat = y.rearrange("d r x -> d (r x)")

    with tile.TileContext(nc) as tc:
        with ExitStack() as ctx:
            xp = ctx.enter_context(tc.tile_pool(name="xin", bufs=1))
            wp = ctx.enter_context(tc.tile_pool(name="wt", bufs=1))
            bp = ctx.enter_context(tc.tile_pool(name="bias", bufs=8))
            op = ctx.enter_context(tc.tile_pool(name="out", bufs=3))
            pp = ctx.enter_context(tc.tile_pool(name="ps", bufs=2, space="PSUM"))

            # PE warmup source: zeroed tile, no DMA dependency
            wdum = wp.tile([128, 640], bf16)
            nc.gpsimd.memset(wdum[:], 0.0)

            # weights + first two band rows gate the first real matmul:
            # first on the sync ring
            wt = wp.tile([128, WCOL + 3 * W], bf16)
            nc.sync.dma_start(wt[:], w[:, :])

            # rest of the input band, front-loaded on the scalar ring
            # (row 1 is re-loaded into xin: wave0's ky=1 taps read it there)
            xin = xp.tile([128, IBAND * W], bf16)
            bounds = [1, 5, 11, 19, 27, IBAND]
            for ci in range(len(bounds) - 1):
                r0, r1 = bounds[ci], bounds[ci + 1]
                nc.scalar.dma_start(xin[:, r0 * W:r1 * W], x[:, r0 * W:r1 * W])

            # all 16 bias tiles resident -> every load enqueues with no wait
            bias_tiles = []
            for g in range(NGRP):
                ra, rb = g * 4, HALF + g * 4
                ba = bp.tile([128, 4 * OW], bf16, name=f"ba{g}", tag="ba")
                nc.sync.dma_start(ba[:], b_flat[:, ra * OW:(ra + 4) * OW])
                bb = bp.tile([128, 4 * OW], bf16, name=f"bb{g}", tag="bb")
                nc.sync.dma_start(bb[:], b_flat[:, rb * OW:(rb + 4) * OW])
                bias_tiles.append((ba, bb))

            # warmup: keep the PE busy (and ramped) while weights land
            pdum = pp.tile([128, 2, 512], f32, name="pdum", tag="pa")
            for i in range(NDUMMY):
                nc.tensor.matmul(
                    pdum[:, 0, 0:OW], wdum[0:128, 512:640], wdum[0:128, 0:510],
                    start=(i == 0), stop=(i == NDUMMY - 1),
                )

            ya = yb = None
            for wv in range(NWAVE):
                g, half = divmod(wv, 2)
                j0 = 2 * wv                    # strip-local output rows
                ba, bb = bias_tiles[g]
                if half == 0:
                    ya = op.tile([128, 4 * OW], bf16, name=f"ya{g}", tag="ya")
                    yb = op.tile([128, 4 * OW], bf16, name=f"yb{g}", tag="yb")

                pa = pp.tile([128, 2, 512], f32, name="pa", tag="pa")
                pb = pp.tile([128, 2, 512], f32, name="pb", tag="pb")
                for t in range(9):
                    ky, kx = divmod(t, 3)
                    if wv == 0 and ky < 2:
                        # rows 0-2 arrive packed behind the weights
                        src = wt
                        o0 = WCOL + ky * W + kx
                        o1 = WCOL + (ky + 1) * W + kx
                    else:
                        src = xin
                        o0 = (j0 + ky) * W + kx
                        o1 = (j0 + 1 + ky) * W + kx
                    st, sp = (t == 0), (t == 8)
                    ws = wt[0:64, t * D:(t + 1) * D]
                    nc.tensor.matmul(pa[:, 0, 0:OW], ws, src[0:64, o0:o0 + OW],
                                     start=st, stop=sp)
                    nc.tensor.matmul(pa[:, 1, 0:OW], ws, src[0:64, o1:o1 + OW],
                                     start=st, stop=sp)
                    ws = wt[64:128, t * D:(t + 1) * D]
                    nc.tensor.matmul(pb[:, 0, 0:OW], ws, src[64:128, o0:o0 + OW],
                                     start=st, stop=sp)
                    nc.tensor.matmul(pb[:, 1, 0:OW], ws, src[64:128, o1:o1 + OW],
                                     start=st, stop=sp)

                ss = slice(2 * half * OW, (2 * half + 2) * OW)
                pav = pa[:, :, 0:OW]
                pbv = pb[:, :, 0:OW]
                if wv == NWAVE - 1:
                    # final wave: strip-b first so its ring starts draining
                    # while strip-a is still evacuating
                    nc.vector.tensor_add(yb[:, ss], pbv, bb[:, ss])
                    nc.vector.tensor_add(ya[:, ss], pav, ba[:, ss])
                else:
                    nc.vector.tensor_add(ya[:, ss], pav, ba[:, ss])
                    nc.vector.tensor_add(yb[:, ss], pbv, bb[:, ss])

                ra, rb = g * 4, HALF + g * 4
                if g < NGRP - 1:
                    if half == 1:
                        ea = nc.scalar if g < 6 else nc.sync
                        eb = nc.scalar
                        ea.dma_start(y_flat[:, ra * OW:(ra + 4) * OW], ya[:])
                        eb.dma_start(y_flat[:, rb * OW:(rb + 4) * OW], yb[:])
                else:
                    # last group: store each wave's 2 rows as soon as its
                    # evac lands, one strip per ring
                    r0 = (ra + 2 * half) * OW
                    r1 = (rb + 2 * half) * OW
                    ss = slice(2 * half * OW, (2 * half + 2) * OW)
                    nc.sync.dma_start(y_flat[:, r0:r0 + 2 * OW], ya[:, ss])
                    nc.scalar.dma_start(y_flat[:, r1:r1 + 2 * OW], yb[:, ss])

    ndedup = _dedup_ldweights(nc)
    assert ndedup >= 288, f"ldweights dedup removed only {ndedup}"
    nc.compile()
    return nc


def kernel(input, kernels, biases):
    global LAST_RESULTS
    import ml_dtypes
    if "nc" not in _CACHE:
        _CACHE["nc"] = _build_program()
    nc = _CACHE["nc"]

    xb = np.asarray(input, dtype=np.float32).astype(ml_dtypes.bfloat16)
    w1 = (
        np.ascontiguousarray(np.asarray(kernels, np.float32).transpose(1, 2, 3, 0))
        .reshape(C, 9 * D).astype(ml_dtypes.bfloat16)
    )
    wr = np.ascontiguousarray(np.concatenate([w1, w1], axis=0))
    bb = np.asarray(biases, np.float32).astype(ml_dtypes.bfloat16)

    in_maps = []
    for s in STARTS:
        band = xb[:, s:s + BAND, :]
        xs = np.concatenate(
            [band[:, 0:IBAND, :], band[:, HALF:HALF + IBAND, :]], axis=0
        ).reshape(2 * C, IBAND * W)
        wx = np.concatenate([wr, xs[:, 0:3 * W]], axis=1)
        in_maps.append({
            "x": np.ascontiguousarray(xs),
            "w": np.ascontiguousarray(wx),
            "b": np.ascontiguousarray(bb[:, s:s + RPC, :]),
        })

    res = bass_utils.run_bass_kernel_spmd(
        nc, in_maps, core_ids=list(range(NCORES))
    )
    LAST_RESULTS = res

    out = np.empty((D, OH, OW), np.float32)
    for i, s in enumerate(STARTS):
        out[:, s:s + RPC, :] = np.asarray(res.results[i]["y"], dtype=np.float32)
    return out
